# revision 18
# baseline (speedup 1.0000x reference)
"""Causal multi-head self-attention (b=4, s=2048, d_model=1024, 16 heads) on 8
Trainium2 NeuronCores.

Sharding: core c handles batch c//2 and head-group c%2 (8 of 16 heads):
wqkv row-split by head (tensor parallel), wo column-split; the host sums the
two partials of each batch while unsharding.

v2 design (ACT-engine exp is the span floor ~195us; everything else hides
under it):
  - All inputs SBUF-resident: x^T, wqkv^T, wo^T, cos/sin tables loaded once at
    t=0 across three DMA rings (sync/gpsimd/scalar). No per-phase reloads.
  - Projections (V token-major; Q/K feature-major with RoPE as cos-mult +
    sin-mult + SWDGE swap-add) are emitted as 8-matmul "groups" that are
    hand-interleaved between attention k-tiles, so the PE has dense work
    while the scalar engine streams exp()s.
  - Attention per (head pair, q-chunk of 512) over causal k-tiles: row-tiled
    score matmul pair (heads at partition halves), causal mask via ident@mtri
    accumulate, one exp per k-tile covering both heads (live columns only),
    AV with a ones-column in V giving softmax denominators for free.
  - Denominator chain stays off the ACT engine: DVE drains PSUM row 64, DMA
    restacks to 2 partitions, one K=2 block-diag ones matmul broadcasts, DVE
    approx-reciprocal, normalize fused into the PSUM drain of y^T.
  - Output projection (head-pair accumulation) pipelined into the last
    attention phase; bf16 partials DMA'd out, host sums pairs (TP all-reduce).
"""

import sys

if "/opt/trn_rl_repo" not in sys.path:
    sys.path.insert(0, "/opt/trn_rl_repo")

from contextlib import ExitStack

import numpy as np

import concourse.bass as bass  # noqa: F401
import concourse.tile as tile
from concourse import bacc, mybir
from concourse.bass_utils import run_bass_kernel_spmd

F32 = mybir.dt.float32
F32R = mybir.dt.float32r
BF16 = mybir.dt.bfloat16
EXP = mybir.ActivationFunctionType.Exp
MULT = mybir.AluOpType.mult
ADD = mybir.AluOpType.add

B, S, D = 4, 2048, 1024
NH_CORE = 8      # heads per core
DH = 64          # head dim
P = 128
TCH = 512        # q/t chunk size
N_HP = NH_CORE // 2
NEG = -1.0e30
ROPE_THETA = 10000.0
SCALE = 1.0 / 8.0  # 1/sqrt(DH)

_CACHE = {}


def _emit(nc, tc, xTp, wqkp, wvp, wop, cosp, sinp, mtri, ident, ones2, outp):
    mm = nc.tensor.matmul
    n_kt = S // P  # 16

    with ExitStack() as ctx:
        # ---------------- persistent (resident) buffers ----------------
        persist = ctx.enter_context(tc.tile_pool(name="persist", bufs=1))
        xT_sb = persist.tile([P, 8, S], BF16, tag="xT", name="xT_sb")
        wqk_sb = persist.tile([P, 8, 1024], BF16, tag="wqk", name="wqk_sb")
        wv_sb = persist.tile([P, 8, 512], BF16, tag="wv", name="wv_sb")
        wo_sb = persist.tile([P, 4, 1024], BF16, tag="wo", name="wo_sb")
        cos_sb = persist.tile([P, S], F32, tag="cos", name="cos_sb")
        sin_sb = persist.tile([P, S], F32, tag="sin", name="sin_sb")
        qkT = [
            persist.tile([P, S], BF16, tag=f"qkT{ft}", name=f"qkT{ft}")
            for ft in range(8)
        ]
        vbuf = persist.tile([P, n_kt, NH_CORE, DH + 1], BF16, tag="vbuf", name="vbuf")
        yT = [
            persist.tile([P, S], BF16, tag=f"yT{hp}", name=f"yT{hp}")
            for hp in range(N_HP)
        ]
        ident_sb = persist.tile([P, P], BF16, tag="ident", name="ident_sb")
        mtri_sb = persist.tile([P, P], BF16, tag="mtri", name="mtri_sb")
        ones2_sb = persist.tile([2, P], F32R, tag="ones2", name="ones2_sb")

        # ---------------- initial loads (3 rings in parallel) ----------------
        nc.gpsimd.dma_start(wqk_sb[:], wqkp.ap()[:, :, :])
        for tc4 in range(4):
            tsl = slice(tc4 * TCH, (tc4 + 1) * TCH)
            nc.sync.dma_start(xT_sb[:, :, tsl], xTp[tc4].ap()[:, :, :])
        nc.scalar.dma_start(wv_sb[:], wvp.ap()[:, :, :])
        nc.scalar.dma_start(mtri_sb[:], mtri.ap()[:, :])
        nc.scalar.dma_start(ident_sb[:], ident.ap()[:, :])
        nc.scalar.dma_start(ones2_sb[:], ones2.ap()[:, :])
        nc.scalar.dma_start(cos_sb[:], cosp.ap()[:, :])
        nc.scalar.dma_start(sin_sb[:], sinp.ap()[:, :])
        nc.scalar.dma_start(wo_sb[:], wop.ap()[:, :, :])
        nc.vector.memset(vbuf[:, :, :, DH : DH + 1], 1.0)

        # ---------------- SBUF working pools ----------------
        btpool = ctx.enter_context(tc.tile_pool(name="btmp", bufs=2))
        epool = ctx.enter_context(tc.tile_pool(name="expS", bufs=10))
        dpool = ctx.enter_context(tc.tile_pool(name="denst", bufs=2))
        rpool = ctx.enter_context(tc.tile_pool(name="recb", bufs=2))

        # ---------------- projection group helpers ----------------
        def b_group(g, pj):
            # V projection for t-tile g (token-major), ones col via memset.
            vps = pj.tile([P, TCH], F32, tag="pj", name="vps")
            for ec in range(8):
                mm(
                    vps[:],
                    xT_sb[:, ec, g * P : (g + 1) * P],
                    wv_sb[:, ec, :],
                    start=(ec == 0),
                    stop=(ec == 7),
                )
            nc.vector.tensor_copy(vbuf[:, g, :, 0:DH], vps[:])

        bt_live = {}

        def a_group(ft, tci, pj):
            # Q/K projection + rope for f-tile ft, token chunk tci. The
            # sin-products accumulate into a full-row btf; the SWDGE swap-adds
            # run once per f-tile over all 2048 columns after the last chunk
            # (full-row 4KB descriptors -- per-descriptor cost dominates).
            tsl = slice(tci * TCH, (tci + 1) * TCH)
            ps = pj.tile([P, TCH], F32, tag="pj", name="ps")
            for ec in range(8):
                mm(
                    ps[:],
                    wqk_sb[:, ec, ft * P : (ft + 1) * P],
                    xT_sb[:, ec, tsl],
                    start=(ec == 0),
                    stop=(ec == 7),
                )
            if tci == 0:
                bt_live[ft] = btpool.tile([P, S], BF16, tag="bt", name="btf")
            btf = bt_live[ft]
            nc.vector.tensor_tensor(qkT[ft][:, tsl], ps[:], cos_sb[:, tsl], MULT)
            nc.vector.tensor_tensor(btf[:, tsl], ps[:], sin_sb[:, tsl], MULT)
            if tci == 3:
                for blk in range(4):
                    a = blk * 32
                    c2 = a ^ 32  # partner half within the 64-row head block
                    nc.gpsimd.dma_start(
                        qkT[ft][c2 : c2 + 32, :], btf[a : a + 32, :], accum_op=ADD
                    )

        # ---------------- attention chunk (software pipelined) ----------------
        LEAD = 4  # k-tiles of scores+exp emitted ahead of their AV matmuls

        def c_chunk(hp, qci, s_ps, av_ps, bc_ps, ktile_hook, prev_tail):
            # bc_ps: pool supplying the [128, 512] f32 PSUM tile for the
            # denominator-broadcast matmul (shared with proj/output pools).
            # Emits LEAD k-tiles of scores+exp first, then the previous
            # chunk's denominator tail, then the AV stream interleaved with
            # the remaining scores+exp -- so the ACT engine keeps streaming
            # exps across the avp-drain chunk boundary (av_ps bufs=1).
            qt = qkT[hp]
            ktt = qkT[4 + hp]
            h0, h1 = 2 * hp, 2 * hp + 1
            qsl = slice(qci * TCH, (qci + 1) * TCH)
            nkt = 4 * qci + 4
            avp = av_ps.tile([DH + 1, 2 * TCH], F32, tag="avp", name="avp")
            elive = {}

            def scores_exp(ki):
                ksl = slice(ki * P, (ki + 1) * P)
                diag = ki >= 4 * qci
                j = ki - 4 * qci
                off = j * P if diag else 0
                qlive = slice(qci * TCH + off, (qci + 1) * TCH)
                sp = s_ps.tile([P, 2 * TCH], F32, tag="sp", name="sp")
                mm(sp[:, off:TCH], ktt[0:64, ksl], qt[0:64, qlive], start=True, stop=True)
                mm(
                    sp[:, TCH + off : 2 * TCH],
                    ktt[64:128, ksl],
                    qt[64:128, qlive],
                    start=True,
                    stop=True,
                )
                if diag:
                    mm(
                        sp[:, off : off + P],
                        ident_sb[:],
                        mtri_sb[:],
                        start=False,
                        stop=True,
                        skip_group_check=True,
                    )
                    mm(
                        sp[:, TCH + off : TCH + off + P],
                        ident_sb[:],
                        mtri_sb[:],
                        start=False,
                        stop=True,
                        skip_group_check=True,
                    )
                e = epool.tile([P, 2 * TCH], BF16, tag="e", name="e")
                sp3 = sp[:].rearrange("p (h q) -> p h q", h=2)
                e3 = e[:].rearrange("p (h q) -> p h q", h=2)
                nc.scalar.activation(e3[:, :, off:], sp3[:, :, off:], EXP, scale=SCALE)
                elive[ki] = (e, off)

            def av(ki):
                e, off = elive.pop(ki)
                mm(
                    avp[:, off:TCH],
                    vbuf[:, ki, h0, :],
                    e[:, off:TCH],
                    start=(ki == 0),
                    stop=(ki == nkt - 1),
                    skip_group_check=True,
                )
                mm(
                    avp[:, TCH + off : 2 * TCH],
                    vbuf[:, ki, h1, :],
                    e[:, TCH + off : 2 * TCH],
                    start=(ki == 0),
                    stop=(ki == nkt - 1),
                    skip_group_check=True,
                )

            for ki in range(min(LEAD, nkt)):
                scores_exp(ki)
            if prev_tail is not None:
                prev_tail()
            for ki in range(nkt):
                av(ki)
                if ki + LEAD < nkt:
                    scores_exp(ki + LEAD)
                ktile_hook()
            # denominator chain head: drain PSUM row 64, restack to 2 parts
            den0 = dpool.tile([1, TCH], F32R, tag="den", name="den0")
            den1 = dpool.tile([1, TCH], F32R, tag="den", name="den1")
            nc.vector.tensor_copy(den0[:], avp[DH : DH + 1, 0:TCH])
            nc.vector.tensor_copy(den1[:], avp[DH : DH + 1, TCH : 2 * TCH])
            den2 = dpool.tile([2, TCH], F32R, tag="den2", name="den2")
            nc.sync.dma_start(den2[0:1, :], den0[:])
            nc.sync.dma_start(den2[1:2, :], den1[:])

            def tail():
                rb = bc_ps.tile([P, TCH], F32, tag="pj", name="rb")
                mm(rb[:], ones2_sb[:, :], den2[:], start=True, stop=True)
                rec = rpool.tile([P, TCH], F32, tag="rec", name="rec")
                rscr = rpool.tile([P, TCH], F32, tag="rscr", name="rscr")
                nc.vector.reciprocal_approx_accurate(rec[:], rb[:], rscr[:])
                nc.vector.tensor_tensor(
                    yT[hp][0:64, qsl], avp[0:DH, 0:TCH], rec[0:64, :], MULT
                )
                nc.vector.tensor_tensor(
                    yT[hp][64:128, qsl], avp[0:DH, TCH : 2 * TCH], rec[64:128, :], MULT
                )

            return tail

        # ---------------- output projection group ----------------
        def d_group(tti, jc, o_ps, osb, ring):
            tsl = slice(tti * P, (tti + 1) * P)
            jsl = slice(jc * TCH, (jc + 1) * TCH)
            op = o_ps.tile([P, TCH], F32, tag="pj", name="op")
            for cc in range(4):
                mm(
                    op[:],
                    yT[cc][:, tsl],
                    wo_sb[:, cc, jsl],
                    start=(cc == 0),
                    stop=(cc == 3),
                )
            ot = osb.tile([P, TCH], BF16, tag="ot", name="ot")
            nc.vector.tensor_copy(ot[:], op[:])
            ring.dma_start(outp.ap()[tsl, jsl], ot[:])

        # ---------------- prologue: A0 first (gates first exp), then B ----
        with ExitStack() as pro:
            pj0 = pro.enter_context(tc.tile_pool(name="pj0", bufs=2, space="PSUM"))
            for ft in (0, 4):
                for tci in range(4):
                    a_group(ft, tci, pj0)
            for g in range(4):
                b_group(g, pj0)

        # ---------------- main: C with interleaved proj groups ----------------
        with ExitStack() as cs:
            s_ps = cs.enter_context(tc.tile_pool(name="s_ps", bufs=2, space="PSUM"))
            av_ps = cs.enter_context(tc.tile_pool(name="av_ps", bufs=1, space="PSUM"))
            pj_stack = ExitStack()
            pjC = pj_stack.enter_context(
                tc.tile_pool(name="pjC", bufs=2, space="PSUM")
            )

            from collections import deque

            pendB = deque(range(4, 16))
            pendA = {
                h: deque((ft, tci) for ft in (h, 4 + h) for tci in range(4))
                for h in (1, 2, 3)
            }
            emittedB = [4]

            def emit_next():
                if pendB:
                    b_group(pendB.popleft(), pjC)
                    emittedB[0] += 1
                    return True
                for h in (1, 2, 3):
                    if pendA[h]:
                        ft, tci = pendA[h].popleft()
                        a_group(ft, tci, pjC)
                        return True
                return False

            def force_b(nkt):
                while emittedB[0] < nkt:
                    b_group(pendB.popleft(), pjC)
                    emittedB[0] += 1

            def force_a(h):
                while pendA[h]:
                    ft, tci = pendA[h].popleft()
                    a_group(ft, tci, pjC)

            kglob = [0]

            def ktile_hook():
                kglob[0] += 1
                if kglob[0] % 2 == 0:
                    emit_next()

            tail = None
            for hp in range(3):
                for h in range(1, hp + 1):
                    force_a(h)
                for qci in range(4):
                    force_b(4 * qci + 4)
                    tail = c_chunk(hp, qci, s_ps, av_ps, pjC, ktile_hook, tail)

            # last head pair: free the proj bank, open output-proj PSUM
            force_a(3)
            tail()  # (2,3)'s denominator tail -- needs pjC, flush before close
            tail = None
            pj_stack.close()
            o_ps = cs.enter_context(tc.tile_pool(name="o_ps", bufs=1, space="PSUM"))
            osb = cs.enter_context(tc.tile_pool(name="osb", bufs=4))

            nohook = lambda: None  # noqa: E731
            d_next = [0]

            def emit_d(upto_tti):
                while d_next[0] < upto_tti * 2:
                    tti, jc = divmod(d_next[0], 2)
                    d_group(tti, jc, o_ps, osb, nc.sync)
                    d_next[0] += 1

            tail = c_chunk(3, 0, s_ps, av_ps, o_ps, nohook, tail)
            tail = c_chunk(3, 1, s_ps, av_ps, o_ps, nohook, tail)
            emit_d(2)
            tail = c_chunk(3, 2, s_ps, av_ps, o_ps, nohook, tail)
            emit_d(6)
            tail = c_chunk(3, 3, s_ps, av_ps, o_ps, nohook, tail)
            tail()

        # ---------------- tail of output projection ----------------
        with ExitStack() as ds:
            o2 = ds.enter_context(tc.tile_pool(name="o2", bufs=3, space="PSUM"))
            osb2 = ds.enter_context(tc.tile_pool(name="osb2", bufs=4))
            rings = [nc.sync, nc.scalar]
            for tti in range(6, n_kt):
                for jc in range(2):
                    d_group(tti, jc, o2, osb2, rings[tti % 2])


def _build():
    key = "nc_v2"
    if key in _CACHE:
        return _CACHE[key]
    nc = bacc.Bacc("TRN2", target_bir_lowering=False, debug=False, num_devices=8)
    xTp = [
        nc.dram_tensor(f"xTp{i}", [P, 8, TCH], BF16, kind="ExternalInput")
        for i in range(4)
    ]
    wqkp = nc.dram_tensor("wqkp", [P, 8, 1024], BF16, kind="ExternalInput")
    wvp = nc.dram_tensor("wvp", [P, 8, 512], BF16, kind="ExternalInput")
    wop = nc.dram_tensor("wop", [P, 4, 1024], BF16, kind="ExternalInput")
    cosp = nc.dram_tensor("cosp", [P, S], F32, kind="ExternalInput")
    sinp = nc.dram_tensor("sinp", [P, S], F32, kind="ExternalInput")
    mtri = nc.dram_tensor("mtri", [P, P], BF16, kind="ExternalInput")
    ident = nc.dram_tensor("ident", [P, P], BF16, kind="ExternalInput")
    ones2 = nc.dram_tensor("ones2", [2, P], F32R, kind="ExternalInput")
    outp = nc.dram_tensor("outp", [S, D], BF16, kind="ExternalOutput")
    with tile.TileContext(nc) as tc:
        _emit(nc, tc, xTp, wqkp, wvp, wop, cosp, sinp, mtri, ident, ones2, outp)
    nc.compile()
    _CACHE[key] = nc
    return nc


def host_inputs(x, wqkv, wo, token_positions):
    """Build the 8 per-core input maps (host-side sharding / layout prep)."""
    import ml_dtypes

    x = np.asarray(x, dtype=np.float32)
    wqkv = np.asarray(wqkv, dtype=np.float32)
    wo = np.asarray(wo, dtype=np.float32)
    pos = np.asarray(token_positions).astype(np.float32)

    d_model = x.shape[2]
    wq, wk, wv = wqkv[0:d_model], wqkv[d_model : 2 * d_model], wqkv[2 * d_model :]

    inv = np.float32(ROPE_THETA) ** (
        -np.arange(0, DH, 2, dtype=np.float32) / np.float32(DH)
    )  # [32]
    ang = pos[None, :] * inv[:, None]  # [32, S]
    cos32 = np.cos(ang).astype(np.float32)
    sin32 = np.sin(ang).astype(np.float32)
    cosp = np.ascontiguousarray(np.tile(cos32, (4, 1)))  # [128, S]
    sinp = np.ascontiguousarray(
        np.tile(np.concatenate([sin32, -sin32], axis=0), (2, 1))
    )  # [128, S]

    a = np.arange(P)
    mtri = np.where(a[:, None] > a[None, :], np.float32(NEG), np.float32(0.0))
    mtri = mtri.astype(ml_dtypes.bfloat16)
    ident = np.eye(P, dtype=ml_dtypes.bfloat16)
    ones2 = np.zeros((2, P), np.float32)
    ones2[0, 0:64] = 1.0
    ones2[1, 64:128] = 1.0

    perm64 = np.concatenate([np.arange(0, DH, 2), np.arange(1, DH, 2)])

    def pmajor(mat, eo):
        # [eo*128, f] -> [128, eo, f]
        return np.ascontiguousarray(
            mat.reshape(eo, P, mat.shape[1]).transpose(1, 0, 2)
        ).astype(ml_dtypes.bfloat16)

    in_maps = []
    for ci in range(8):
        bi, hg = divmod(ci, 2)
        xT = np.ascontiguousarray(x[bi].T)  # [1024, 2048]
        xTr = pmajor(xT, 8)  # [128, 8, 2048]
        rows = []
        for blk in (wq, wk):
            for h in range(hg * NH_CORE, (hg + 1) * NH_CORE):
                rows.append(blk[h * DH : (h + 1) * DH][perm64])
        wqkT = np.ascontiguousarray(np.concatenate(rows, axis=0).T)  # [1024, 1024]
        wvT = np.ascontiguousarray(wv[hg * 512 : (hg + 1) * 512].T)  # [1024, 512]
        woT = np.ascontiguousarray(wo[:, hg * 512 : (hg + 1) * 512].T)  # [512, 1024]
        m = {
            "wqkp": pmajor(wqkT, 8),
            "wvp": pmajor(wvT, 8),
            "wop": pmajor(woT, 4),
            "cosp": cosp,
            "sinp": sinp,
            "mtri": mtri,
            "ident": ident,
            "ones2": ones2,
        }
        for i in range(4):
            m[f"xTp{i}"] = np.ascontiguousarray(xTr[:, :, i * TCH : (i + 1) * TCH])
        in_maps.append(m)
    return in_maps


def _install_ntff_hook():
    """Recreate the antenv.axon_hooks NTFF profile hook this image lacks
    (same ctypes shim trn_agent_boot would register). Dev/profiling only."""
    import contextlib
    import ctypes
    import os
    import types

    try:
        import antenv.axon_hooks  # noqa: F401

        return
    except ImportError:
        pass
    so_path = "/opt/axon/libaxon_pjrt.so"
    if not os.path.exists(so_path):
        return
    lib = ctypes.CDLL(so_path)
    if not hasattr(lib, "axon_start_nrt_profile"):
        return
    lib.axon_start_nrt_profile.argtypes = [
        ctypes.POINTER(ctypes.c_int64),
        ctypes.c_size_t,
    ]
    lib.axon_start_nrt_profile.restype = ctypes.c_int64
    lib.axon_stop_nrt_profile.argtypes = [ctypes.c_char_p]
    lib.axon_stop_nrt_profile.restype = ctypes.c_int64

    @contextlib.contextmanager
    def _hook(output_dir, device_ids):
        import jax

        jax.devices()
        if device_ids:
            ids = (ctypes.c_int64 * len(device_ids))(*device_ids)
            rc = lib.axon_start_nrt_profile(ids, len(device_ids))
        else:
            rc = lib.axon_start_nrt_profile(None, 0)
        if rc != 0:
            raise RuntimeError(f"axon_start_nrt_profile rc={rc}")
        try:
            yield
        finally:
            n = lib.axon_stop_nrt_profile(str(output_dir).encode())
            if n < 0:
                raise RuntimeError(f"axon_stop_nrt_profile rc={n}")

    import antenv
    from concourse import bass_utils as _bu

    _bu.upload_artifacts = lambda d: d  # no bucket access in this container
    mod = types.ModuleType("antenv.axon_hooks")
    mod.get_axon_ntff_profile_hook = lambda: _hook
    mod.set_axon_ntff_profile_hook = lambda h: None
    sys.modules["antenv.axon_hooks"] = mod
    antenv.axon_hooks = mod


def kernel(x, wqkv, wo, token_positions, trace=False):
    if trace:
        _install_ntff_hook()
    nc = _build()
    in_maps = host_inputs(x, wqkv, wo, token_positions)
    res = run_bass_kernel_spmd(nc, in_maps, core_ids=list(range(8)), trace=trace)
    parts = [np.asarray(res.results[ci]["outp"]).astype(np.float32) for ci in range(8)]
    out = np.stack([parts[2 * bi] + parts[2 * bi + 1] for bi in range(B)], axis=0)
    if trace:
        kernel.last_result = res
    return out
